# revision 27
# baseline (speedup 1.0000x reference)
"""Trainium2 Bass kernel for nn_Attention: 8-head attention block, data-parallel
over batch across 8 NeuronCores (one batch element per core, no collectives).

kernel(**inputs) takes the full unsharded inputs and returns (O, Am) matching
the reference:
    O  [8, 1024, 512] f32
    Am [8, 1024, 1024] f32   (head-mean attention, transposed to [B, Nk, Nq])

Per-core algorithm (batch element b). Matmuls run in float32r (single-pass PE
mode, ~1e-4 rounding, 4x the fp32 rate); use_f32r=False selects exact 2-pass
fp32. Everything lives in "transposed" token layout so no attention-matrix
transposes are ever needed:
  QT/KT   <- PE-transpose of Q[b], K[b]                     [d_in, tok]
  qT, kT  <- Wq/Wk stationary @ QT/KT (+bias, per-partition) [d_out, tok]
  v_pad   <- KT-chunks stationary @ Wv (+bias via rank-1 ones matmul),
             stored per-head padded with an 8.0-column       [tok, 8*(64+1)]
  per head:
    E^T[k,q] <- exp(scale * khT' @ qhT) per k-tile: the scores matmul is
             emitted pre-transposed and the ScalarE exp evict materializes
             E^T directly in SBUF (no PE transposes, no extra PSUM evicts)
    AV      <- v_pad-slice stationary @ E^T; the 8.0-column makes PSUM row 64
             hold 8*rowsum, so softmax normalization comes free: reciprocal
             of that row gives r/8, broadcast via a DRAM-bounce DMA
    OhT     <- psum * (r/8-bcast) * 8 + qhT residual (fused DVE ops)
    Am^T    += in-place E^T * (r/8-bcast)  (muls/adds split across GPSIMD/DVE;
             E^T double-buffered across heads to keep all engines pipelined)
  final^T  <- Wo stationary @ OT; ReLU+bias on ScalarE; + OT residual
  O        <- PE-transpose back to natural layout
"""
import sys

sys.path.insert(0, "/opt/trn_rl_repo")

import math
from contextlib import ExitStack

import numpy as np

import concourse.bass as bass
import concourse.mybir as mybir
import concourse.tile as tile
from concourse import bacc
from concourse.bass_utils import run_bass_kernel_spmd

N_CORES = 8
B = 8
N = 1024          # tokens (Nq == Nk)
D = 512           # model dim
H = 8             # heads
DH = D // H       # 64
P = 128           # partitions
NT = N // P       # 8 token tiles
DC = D // P       # 4 dim chunks
SCALE = 1.0 / math.sqrt(D)

F32 = mybir.dt.float32

_MODULE_CACHE = {}


def build_module(use_f32r=True):
    """Build + compile the per-core Bass module.

    use_f32r: run PE matmuls in float32r (single-pass, 4x faster, ~1e-4
    rounding) instead of exact 2-pass float32.
    """
    mm_dt = mybir.dt.float32r if use_f32r else F32

    def mk(ap):
        return ap.bitcast(mm_dt) if use_f32r else ap

    nc = bacc.Bacc("TRN2", target_bir_lowering=False, debug=False,
                   num_devices=N_CORES)

    Qd = nc.dram_tensor("Q", [N, D], F32, kind="ExternalInput")
    Kd = nc.dram_tensor("K", [N, D], F32, kind="ExternalInput")
    Wts = {w: nc.dram_tensor(w, [D, D], F32, kind="ExternalInput")
           for w in ("Wq", "Wk", "Wv", "Wo")}
    Bs = {b: nc.dram_tensor(b, [D], F32, kind="ExternalInput")
          for b in ("bq", "bk", "bv", "bo")}
    Od = nc.dram_tensor("O", [N, D], F32, kind="ExternalOutput")
    Amd = nc.dram_tensor("Am", [N, N], F32, kind="ExternalOutput")

    ident_dram = nc.inline_tensor(np.eye(P, dtype=np.float32), name="identc")
    eights_dram = nc.inline_tensor(np.full((P, H), 8.0, dtype=np.float32), name="eightsc")
    ones_dram = nc.inline_tensor(np.ones((1, P), dtype=np.float32), name="onesc")

    def cast_dma(out, in_):
        # fp32r consumers need an fp32r-typed producer; HW rounds internally,
        # so a bitcast on both sides (raw bytes, HWDGE) is sufficient.
        nc.sync.dma_start(out=mk(out), in_=mk(in_))

    with tile.TileContext(nc) as tc, ExitStack() as top:
        consts = top.enter_context(tc.tile_pool(name="consts", bufs=1))
        persist = top.enter_context(tc.tile_pool(name="persist", bufs=1))

        # ---- constants (ident first: stage B needs it immediately) ----
        ident = consts.tile([P, P], F32, tag="ident")
        cast_dma(ident, ident_dram.ap())
        ones_col = consts.tile([1, P], F32, tag="ones")
        bv_row = consts.tile([1, D], F32, tag="bvr")
        bias_t = {name: consts.tile([P, DC], F32, tag=f"{name}_t", name=f"{name}_t")
                  for name in ("bq", "bk", "bo")}
        w_sb = {"Wo": [consts.tile([P, D], F32, tag=f"Wo_{kc}", name=f"Wo_{kc}")
                       for kc in range(DC)]}

        def load_weights():
            # emitted after the Q/K loads so input DMAs win the queues early
            cast_dma(ones_col, ones_dram.ap())
            cast_dma(bv_row, Bs["bv"].ap().unsqueeze(0))
            for name in ("bq", "bk", "bo"):
                nc.sync.dma_start(out=bias_t[name],
                                  in_=Bs[name].ap().rearrange("(c p) -> p c", p=P))
            for kc in range(DC):
                cast_dma(w_sb["Wo"][kc], Wts["Wo"].ap()[kc * P:(kc + 1) * P, :])
            for name in ("Wq", "Wk", "Wv"):
                for kc in range(DC):
                    cast_dma(w_sb[name][kc], Wts[name].ap()[kc * P:(kc + 1) * P, :])

        # ---- persistent activations (live across stages) ----
        qT = [persist.tile([P, N], F32, tag=f"qT{c}", name=f"qT{c}") for c in range(DC)]
        kT = [persist.tile([P, N], F32, tag=f"kT{c}", name=f"kT{c}") for c in range(DC)]
        VW = H * (DH + 1)  # 520: per-head [64 v-cols | one 8.0-col]
        v_sb = [persist.tile([P, VW], F32, tag=f"v{t}", name=f"v{t}") for t in range(NT)]
        OT = [persist.tile([P, N], F32, tag=f"OT{c}", name=f"OT{c}") for c in range(DC)]

        # ============ Stage B+C: input transposes and projections ============
        with (
            tc.tile_pool(name="ldq", bufs=6) as ldq,
            tc.tile_pool(name="qtp", bufs=1) as qtp,
            tc.tile_pool(name="psb", bufs=4, space="PSUM") as psb,
            tc.tile_pool(name="psc", bufs=4, space="PSUM") as psc,
        ):
            for name in ("Wq", "Wk", "Wv"):
                w_sb[name] = [qtp.tile([P, D], F32, tag=f"{name}_{kc}", name=f"{name}_{kc}")
                              for kc in range(DC)]
            QT = [qtp.tile([P, N], F32, tag=f"QT{c}", name=f"QT{c}") for c in range(DC)]
            KT = [qtp.tile([P, N], F32, tag=f"KT{c}", name=f"KT{c}") for c in range(DC)]
            nats = []
            for src, dstT in ((Qd, QT), (Kd, KT)):
                for t in range(NT):
                    nat = ldq.tile([P, D], F32, tag="nat")
                    nc.sync.dma_start(out=nat, in_=src.ap()[t * P:(t + 1) * P, :])
                    nats.append((nat, dstT, t))
            load_weights()
            for nat, dstT, t in nats:
                for dc in range(DC):
                    ps = psb.tile([P, P], F32, tag="ps")
                    nc.tensor.transpose(ps, nat[:, dc * P:(dc + 1) * P], ident)
                    nc.scalar.copy(mk(dstT[dc][:, t * P:(t + 1) * P]), ps)

            for wname, bname, srcT, dstT in (("Wq", "bq", QT, qT),
                                             ("Wk", "bk", KT, kT)):
                for dc in range(DC):
                    for hf in range(2):
                        ps = psc.tile([P, D], F32, tag="psp")
                        for kc in range(DC):
                            nc.tensor.matmul(
                                ps,
                                mk(w_sb[wname][kc][:, dc * P:(dc + 1) * P]),
                                mk(srcT[kc][:, hf * D:(hf + 1) * D]),
                                start=(kc == 0), stop=(kc == DC - 1),
                            )
                        nc.vector.tensor_scalar_add(
                            mk(dstT[dc][:, hf * D:(hf + 1) * D]),
                            ps, bias_t[bname][:, dc:dc + 1])
            for t in range(NT):
                ps = psc.tile([P, D], F32, tag="psp")
                for kc in range(DC):
                    nc.tensor.matmul(
                        ps,
                        mk(KT[kc][:, t * P:(t + 1) * P]),
                        mk(w_sb["Wv"][kc]),
                        start=(kc == 0), stop=False,
                    )
                nc.tensor.matmul(ps, mk(ones_col), mk(bv_row),
                                 start=False, stop=True)
                vp = v_sb[t].rearrange("p (h w) -> p h w", h=H)
                nc.scalar.copy(mk(vp[:, :, 0:DH]),
                               ps.rearrange("p (h d) -> p h d", h=H))
                nc.sync.dma_start(out=mk(vp[:, :, DH:DH + 1].squeeze(2)),
                                  in_=mk(eights_dram.ap()))

        # ========================= Stage D: attention ========================
        # scores computed pre-transposed: scoresT[k, q] = khT' @ qhT.
        # exp evict doubles as E^T materialization; row-sums via ones-matmul;
        # 1/s broadcast via DRAM bounce; normalize + Am-accumulate on GPSIMD.
        with (
            tc.tile_pool(name="attbuf", bufs=1) as attbuf,
            tc.tile_pool(name="attsm", bufs=6) as attsm,
            tc.tile_pool(name="rbc", bufs=3) as rbc,
            tc.tile_pool(name="pss", bufs=2, space="PSUM") as pss,
            tc.tile_pool(name="pso", bufs=2, space="PSUM") as pso,
            tc.tile_pool(name="rps", bufs=1, space="PSUM") as rps,
        ):
            AmT = [attbuf.tile([P, N], F32, tag=f"AmT{k}", name=f"AmT{k}") for k in range(NT)]
            ET_sets = [[attbuf.tile([P, N], F32, tag=f"ET{s}_{k}", name=f"ET{s}_{k}")
                        for k in range(NT)] for s in range(2)]
            for h in range(H):
                hc, hr = h // 2, (h % 2) * DH
                ET = ET_sets[h % 2]
                for kt in range(NT):
                    ps = pss.tile([P, N], F32, tag="pss")
                    for hf in range(2):
                        nc.tensor.matmul(
                            ps[:, hf * D:(hf + 1) * D],
                            mk(kT[hc][hr:hr + DH, kt * P:(kt + 1) * P]),
                            mk(qT[hc][hr:hr + DH, hf * D:(hf + 1) * D]),
                            start=True, stop=True,
                        )
                    nc.scalar.activation(
                        out=mk(ET[kt]), in_=ps,
                        func=mybir.ActivationFunctionType.Exp,
                        scale=SCALE)
                # AV on unnormalized E^T; v_pad's 8.0-column makes psum row 64
                # hold 8*rowsum, so reciprocal gives r/8 directly.
                r_row = attsm.tile([1, N], F32, tag="r_row")
                ps_os = []
                for hf in range(2):
                    ps_o = pso.tile([DH + 1, D], F32, tag="pso")
                    ps_os.append(ps_o)
                    for kt in range(NT):
                        nc.tensor.matmul(
                            ps_o,
                            mk(v_sb[kt][:, h * (DH + 1):(h + 1) * (DH + 1)]),
                            mk(ET[kt][:, hf * D:(hf + 1) * D]),
                            start=(kt == 0), stop=(kt == NT - 1),
                        )
                    with nc.allow_low_precision(reason="f32r view, same bytes"):
                        nc.vector.reciprocal(mk(r_row[:, hf * D:(hf + 1) * D]),
                                             ps_o[DH:DH + 1, :])
                r_ps = rps.tile([P, N], F32, tag="r_ps")
                for hf in range(2):
                    nc.tensor.matmul(
                        r_ps[:, hf * D:(hf + 1) * D], mk(ones_col),
                        mk(r_row[:, hf * D:(hf + 1) * D]),
                        start=True, stop=True)
                r_bcast = rbc.tile([P, N], F32, tag="r_bcast")
                nc.scalar.copy(r_bcast, r_ps)  # feeds Pool muls + OhT mult
                for hf in range(2):
                    ps_o = ps_os[hf]
                    nc.vector.tensor_mul(
                        ps_o[0:DH, :], ps_o[0:DH, :],
                        r_bcast[0:DH, hf * D:(hf + 1) * D])
                    nc.vector.scalar_tensor_tensor(
                        out=mk(OT[hc][hr:hr + DH, hf * D:(hf + 1) * D]),
                        in0=ps_o[0:DH, :], scalar=8.0,
                        in1=qT[hc][hr:hr + DH, hf * D:(hf + 1) * D],
                        op0=mybir.AluOpType.mult, op1=mybir.AluOpType.add)
                # ET[kt] <- A^T/8 (feeds Am accumulation). All muls emitted
                # before all adds: keeps each engine's in-order queue busy.
                for kt in range(NT):
                    if kt < 5:
                        nc.vector.tensor_mul(mk(ET[kt]), ET[kt], r_ps)
                    else:
                        nc.gpsimd.tensor_mul(mk(ET[kt]), ET[kt], r_bcast)
                for kt in range(NT):
                    # DVE adds only tiles DVE mul'd: no cross-engine stalls
                    eng2 = nc.vector if kt < 4 else nc.gpsimd
                    if h == 0:
                        eng2.tensor_copy(AmT[kt], ET[kt])
                    else:
                        eng2.tensor_add(AmT[kt], AmT[kt], ET[kt])
            for kt in range(NT):
                nc.sync.dma_start(out=Amd.ap()[kt * P:(kt + 1) * P, :], in_=AmT[kt])

        # ============= Stage E+F: output projection & writeback ==============
        with (
            tc.tile_pool(name="fin", bufs=2) as fin,
            tc.tile_pool(name="fre", bufs=4) as fre,
            tc.tile_pool(name="psf", bufs=3, space="PSUM") as psf,
            tc.tile_pool(name="psg", bufs=3, space="PSUM") as psg,
        ):
            finalT = [fin.tile([P, N], F32, tag=f"fT{c}", name=f"fT{c}") for c in range(DC)]
            for dc in range(DC):
                for hf in range(2):
                    ps = psf.tile([P, D], F32, tag="psf")
                    for kc in range(DC):
                        nc.tensor.matmul(
                            ps,
                            mk(w_sb["Wo"][kc][:, dc * P:(dc + 1) * P]),
                            mk(OT[kc][:, hf * D:(hf + 1) * D]),
                            start=(kc == 0), stop=(kc == DC - 1),
                        )
                    relu = fre.tile([P, D], F32, tag="relu")
                    nc.scalar.activation(
                        out=relu, in_=ps, func=mybir.ActivationFunctionType.Relu,
                        bias=bias_t["bo"][:, dc:dc + 1], scale=1.0)
                    nc.vector.tensor_add(
                        finalT[dc][:, hf * D:(hf + 1) * D], relu,
                        OT[dc][:, hf * D:(hf + 1) * D])
            for t in range(NT):
                onat = fre.tile([P, D], F32, tag="onat")
                for dc in range(DC):
                    ps = psg.tile([P, P], F32, tag="psg")
                    nc.tensor.transpose(ps, finalT[dc][:, t * P:(t + 1) * P], ident)
                    nc.scalar.copy(onat[:, dc * P:(dc + 1) * P], ps)
                nc.sync.dma_start(out=Od.ap()[t * P:(t + 1) * P, :], in_=onat)

    nc.compile()
    return nc


def kernel(Q, K, Wq, bq, Wk, bk, Wv, bv, Wo, bo, use_f32r=True):
    key = bool(use_f32r)
    if key not in _MODULE_CACHE:
        _MODULE_CACHE[key] = build_module(use_f32r=key)
    nc = _MODULE_CACHE[key]

    Q = np.ascontiguousarray(np.asarray(Q, dtype=np.float32))
    K = np.ascontiguousarray(np.asarray(K, dtype=np.float32))
    shared = {
        name: np.ascontiguousarray(np.asarray(val, np.float32))
        for name, val in (("Wq", Wq), ("Wk", Wk), ("Wv", Wv), ("Wo", Wo),
                          ("bq", bq), ("bk", bk), ("bv", bv), ("bo", bo))
    }
    in_maps = [dict(shared, Q=Q[b], K=K[b]) for b in range(N_CORES)]
    out = run_bass_kernel_spmd(nc, in_maps, list(range(N_CORES)))
    O = np.stack([out.results[b]["O"] for b in range(N_CORES)])
    Am = np.stack([out.results[b]["Am"] for b in range(N_CORES)])
    return O, Am


# revision 28
# speedup vs baseline: 1.0554x; 1.0554x over previous
"""Trainium2 Bass kernel for nn_Attention: 8-head attention block, data-parallel
over batch across 8 NeuronCores (one batch element per core, no collectives).

kernel(**inputs) takes the full unsharded inputs and returns (O, Am) matching
the reference:
    O  [8, 1024, 512] f32
    Am [8, 1024, 1024] f32   (head-mean attention, transposed to [B, Nk, Nq])

Per-core algorithm (batch element b). Matmuls run in float32r (single-pass PE
mode, ~1e-4 rounding, 4x the fp32 rate); use_f32r=False selects exact 2-pass
fp32. Everything lives in "transposed" token layout so no attention-matrix
transposes are ever needed:
  QT/KT   <- PE-transpose of Q[b], K[b]                     [d_in, tok]
  qT, kT  <- Wq/Wk stationary @ QT/KT (+bias, per-partition) [d_out, tok]
  v_pad   <- KT-chunks stationary @ Wv (+bias via rank-1 ones matmul),
             stored per-head padded with an 8.0-column       [tok, 8*(64+1)]
  per head:
    E^T[k,q] <- exp(scale * khT' @ qhT) per k-tile: the scores matmul is
             emitted pre-transposed and the ScalarE exp evict materializes
             E^T directly in SBUF (no PE transposes, no extra PSUM evicts)
    AV      <- v_pad-slice stationary @ E^T; the 8.0-column makes PSUM row 64
             hold 8*rowsum, so softmax normalization comes free: reciprocal
             of that row gives r/8, broadcast via a DRAM-bounce DMA
    OhT     <- psum * (r/8-bcast) * 8 + qhT residual (fused DVE ops)
    Am^T    += in-place E^T * (r/8-bcast)  (muls/adds split across GPSIMD/DVE;
             E^T double-buffered across heads to keep all engines pipelined)
  final^T  <- Wo stationary @ OT; ReLU+bias on ScalarE; + OT residual
  O        <- PE-transpose back to natural layout
"""
import sys

sys.path.insert(0, "/opt/trn_rl_repo")

import math
from contextlib import ExitStack

import numpy as np

import concourse.bass as bass
import concourse.mybir as mybir
import concourse.tile as tile
from concourse import bacc
from concourse.bass_utils import run_bass_kernel_spmd

N_CORES = 8
B = 8
N = 1024          # tokens (Nq == Nk)
D = 512           # model dim
H = 8             # heads
DH = D // H       # 64
P = 128           # partitions
NT = N // P       # 8 token tiles
DC = D // P       # 4 dim chunks
SCALE = 1.0 / math.sqrt(D)

F32 = mybir.dt.float32

_MODULE_CACHE = {}


def build_module(use_f32r=True):
    """Build + compile the per-core Bass module.

    use_f32r: run PE matmuls in float32r (single-pass, 4x faster, ~1e-4
    rounding) instead of exact 2-pass float32.
    """
    mm_dt = mybir.dt.float32r if use_f32r else F32

    def mk(ap):
        return ap.bitcast(mm_dt) if use_f32r else ap

    nc = bacc.Bacc("TRN2", target_bir_lowering=False, debug=False,
                   num_devices=N_CORES)

    Qd = nc.dram_tensor("Q", [N, D], F32, kind="ExternalInput")
    Kd = nc.dram_tensor("K", [N, D], F32, kind="ExternalInput")
    Wts = {w: nc.dram_tensor(w, [D, D], F32, kind="ExternalInput")
           for w in ("Wq", "Wk", "Wv", "Wo")}
    Bs = {b: nc.dram_tensor(b, [D], F32, kind="ExternalInput")
          for b in ("bq", "bk", "bv", "bo")}
    Od = nc.dram_tensor("O", [N, D], F32, kind="ExternalOutput")
    Amd = nc.dram_tensor("Am", [N, N], F32, kind="ExternalOutput")

    ident_dram = nc.inline_tensor(np.eye(P, dtype=np.float32), name="identc")
    eights_dram = nc.inline_tensor(np.full((P, H), 8.0, dtype=np.float32), name="eightsc")
    ones_dram = nc.inline_tensor(np.ones((1, P), dtype=np.float32), name="onesc")

    def cast_dma(out, in_):
        # fp32r consumers need an fp32r-typed producer; HW rounds internally,
        # so a bitcast on both sides (raw bytes, HWDGE) is sufficient.
        nc.sync.dma_start(out=mk(out), in_=mk(in_))

    with tile.TileContext(nc) as tc, ExitStack() as top:
        consts = top.enter_context(tc.tile_pool(name="consts", bufs=1))
        persist = top.enter_context(tc.tile_pool(name="persist", bufs=1))

        # ---- constants (ident first: stage B needs it immediately) ----
        ident = consts.tile([P, P], F32, tag="ident")
        cast_dma(ident, ident_dram.ap())
        ones_col = consts.tile([1, P], F32, tag="ones")
        bv_row = consts.tile([1, D], F32, tag="bvr")
        bias_t = {name: consts.tile([P, DC], F32, tag=f"{name}_t", name=f"{name}_t")
                  for name in ("bq", "bk", "bo")}
        w_sb = {"Wo": [consts.tile([P, D], F32, tag=f"Wo_{kc}", name=f"Wo_{kc}")
                       for kc in range(DC)]}

        def load_weights():
            # emitted after the Q/K loads so input DMAs win the queues early
            cast_dma(ones_col, ones_dram.ap())
            cast_dma(bv_row, Bs["bv"].ap().unsqueeze(0))
            for name in ("bq", "bk", "bo"):
                nc.sync.dma_start(out=bias_t[name],
                                  in_=Bs[name].ap().rearrange("(c p) -> p c", p=P))
            for kc in range(DC):
                cast_dma(w_sb["Wo"][kc], Wts["Wo"].ap()[kc * P:(kc + 1) * P, :])
            for name in ("Wq", "Wk", "Wv"):
                for kc in range(DC):
                    cast_dma(w_sb[name][kc], Wts[name].ap()[kc * P:(kc + 1) * P, :])

        # ---- persistent activations (live across stages) ----
        qT = [persist.tile([P, N], F32, tag=f"qT{c}", name=f"qT{c}") for c in range(DC)]
        kT = [persist.tile([P, N], F32, tag=f"kT{c}", name=f"kT{c}") for c in range(DC)]
        VW = H * (DH + 1)  # 520: per-head [64 v-cols | one 8.0-col]
        v_sb = [persist.tile([P, VW], F32, tag=f"v{t}", name=f"v{t}") for t in range(NT)]
        OT = [persist.tile([P, N], F32, tag=f"OT{c}", name=f"OT{c}") for c in range(DC)]

        # ============ Stage B+C: input transposes and projections ============
        with (
            tc.tile_pool(name="ldq", bufs=6) as ldq,
            tc.tile_pool(name="qtp", bufs=1) as qtp,
            tc.tile_pool(name="psb", bufs=4, space="PSUM") as psb,
            tc.tile_pool(name="psc", bufs=4, space="PSUM") as psc,
        ):
            for name in ("Wq", "Wk", "Wv"):
                w_sb[name] = [qtp.tile([P, D], F32, tag=f"{name}_{kc}", name=f"{name}_{kc}")
                              for kc in range(DC)]
            QT = [qtp.tile([P, N], F32, tag=f"QT{c}", name=f"QT{c}") for c in range(DC)]
            KT = [qtp.tile([P, N], F32, tag=f"KT{c}", name=f"KT{c}") for c in range(DC)]
            nats = []
            for src, dstT in ((Qd, QT), (Kd, KT)):
                for t in range(NT):
                    nat = ldq.tile([P, D], F32, tag="nat")
                    nc.sync.dma_start(out=nat, in_=src.ap()[t * P:(t + 1) * P, :])
                    nats.append((nat, dstT, t))
            load_weights()
            for nat, dstT, t in nats:
                for dc in range(DC):
                    ps = psb.tile([P, P], F32, tag="ps")
                    nc.tensor.transpose(ps, nat[:, dc * P:(dc + 1) * P], ident)
                    nc.scalar.copy(mk(dstT[dc][:, t * P:(t + 1) * P]), ps)

            for wname, bname, srcT, dstT in (("Wq", "bq", QT, qT),
                                             ("Wk", "bk", KT, kT)):
                for dc in range(DC):
                    for hf in range(2):
                        ps = psc.tile([P, D], F32, tag="psp")
                        for kc in range(DC):
                            nc.tensor.matmul(
                                ps,
                                mk(w_sb[wname][kc][:, dc * P:(dc + 1) * P]),
                                mk(srcT[kc][:, hf * D:(hf + 1) * D]),
                                start=(kc == 0), stop=(kc == DC - 1),
                            )
                        nc.vector.tensor_scalar_add(
                            mk(dstT[dc][:, hf * D:(hf + 1) * D]),
                            ps, bias_t[bname][:, dc:dc + 1])
            for t in range(NT):
                ps = psc.tile([P, D], F32, tag="psp")
                for kc in range(DC):
                    nc.tensor.matmul(
                        ps,
                        mk(KT[kc][:, t * P:(t + 1) * P]),
                        mk(w_sb["Wv"][kc]),
                        start=(kc == 0), stop=False,
                    )
                nc.tensor.matmul(ps, mk(ones_col), mk(bv_row),
                                 start=False, stop=True)
                vp = v_sb[t].rearrange("p (h w) -> p h w", h=H)
                nc.scalar.copy(mk(vp[:, :, 0:DH]),
                               ps.rearrange("p (h d) -> p h d", h=H))
                nc.sync.dma_start(out=mk(vp[:, :, DH:DH + 1].squeeze(2)),
                                  in_=mk(eights_dram.ap()))

        # ========================= Stage D: attention ========================
        # scores computed pre-transposed: scoresT[k, q] = khT' @ qhT.
        # exp evict doubles as E^T materialization; row-sums via ones-matmul;
        # 1/s broadcast via DRAM bounce; normalize + Am-accumulate on GPSIMD.
        with (
            tc.tile_pool(name="attbuf", bufs=1) as attbuf,
            tc.tile_pool(name="attsm", bufs=6) as attsm,
            tc.tile_pool(name="rbc", bufs=3) as rbc,
            tc.tile_pool(name="pss", bufs=2, space="PSUM") as pss,
            tc.tile_pool(name="pso", bufs=2, space="PSUM") as pso,
            tc.tile_pool(name="rps", bufs=1, space="PSUM") as rps,
        ):
            AmT = [attbuf.tile([P, N], F32, tag=f"AmT{k}", name=f"AmT{k}") for k in range(NT)]
            ET_sets = [[attbuf.tile([P, N], F32, tag=f"ET{s}_{k}", name=f"ET{s}_{k}")
                        for k in range(NT)] for s in range(2)]
            for h in range(H):
                hc, hr = h // 2, (h % 2) * DH
                ET = ET_sets[h % 2]
                for kt in range(NT):
                    ps = pss.tile([P, N], F32, tag="pss")
                    for hf in range(2):
                        nc.tensor.matmul(
                            ps[:, hf * D:(hf + 1) * D],
                            mk(kT[hc][hr:hr + DH, kt * P:(kt + 1) * P]),
                            mk(qT[hc][hr:hr + DH, hf * D:(hf + 1) * D]),
                            start=True, stop=True,
                        )
                    nc.scalar.activation(
                        out=mk(ET[kt]), in_=ps,
                        func=mybir.ActivationFunctionType.Exp,
                        scale=SCALE)
                # AV on unnormalized E^T; v_pad's 8.0-column makes psum row 64
                # hold 8*rowsum, so reciprocal gives r/8 directly.
                r_row = attsm.tile([1, N], F32, tag="r_row")
                ps_os = []
                for hf in range(2):
                    ps_o = pso.tile([DH + 1, D], F32, tag="pso")
                    ps_os.append(ps_o)
                    for kt in range(NT):
                        nc.tensor.matmul(
                            ps_o,
                            mk(v_sb[kt][:, h * (DH + 1):(h + 1) * (DH + 1)]),
                            mk(ET[kt][:, hf * D:(hf + 1) * D]),
                            start=(kt == 0), stop=(kt == NT - 1),
                        )
                    with nc.allow_low_precision(reason="f32r view, same bytes"):
                        nc.vector.reciprocal(mk(r_row[:, hf * D:(hf + 1) * D]),
                                             ps_o[DH:DH + 1, :])
                r_ps = rps.tile([P, N], F32, tag="r_ps")
                for hf in range(2):
                    nc.tensor.matmul(
                        r_ps[:, hf * D:(hf + 1) * D], mk(ones_col),
                        mk(r_row[:, hf * D:(hf + 1) * D]),
                        start=True, stop=True)
                r_bcast = rbc.tile([P, N], F32, tag="r_bcast")
                nc.vector.tensor_copy(r_bcast, r_ps)
                for hf in range(2):
                    ps_o = ps_os[hf]
                    nc.vector.tensor_mul(
                        ps_o[0:DH, :], ps_o[0:DH, :],
                        r_bcast[0:DH, hf * D:(hf + 1) * D])
                    nc.vector.scalar_tensor_tensor(
                        out=mk(OT[hc][hr:hr + DH, hf * D:(hf + 1) * D]),
                        in0=ps_o[0:DH, :], scalar=8.0,
                        in1=qT[hc][hr:hr + DH, hf * D:(hf + 1) * D],
                        op0=mybir.AluOpType.mult, op1=mybir.AluOpType.add)
                # ET[kt] <- A^T/8 (feeds Am accumulation). All muls emitted
                # before all adds: keeps each engine's in-order queue busy.
                for kt in range(NT):
                    eng = nc.vector if kt < 5 else nc.gpsimd
                    eng.tensor_mul(mk(ET[kt]), ET[kt], r_bcast)
                for kt in range(NT):
                    # DVE adds only tiles DVE mul'd: no cross-engine stalls
                    eng2 = nc.vector if kt < 4 else nc.gpsimd
                    if h == 0:
                        eng2.tensor_copy(AmT[kt], ET[kt])
                    else:
                        eng2.tensor_add(AmT[kt], AmT[kt], ET[kt])
            for kt in range(NT):
                nc.sync.dma_start(out=Amd.ap()[kt * P:(kt + 1) * P, :], in_=AmT[kt])

        # ============= Stage E+F: output projection & writeback ==============
        with (
            tc.tile_pool(name="fin", bufs=2) as fin,
            tc.tile_pool(name="fre", bufs=4) as fre,
            tc.tile_pool(name="psf", bufs=3, space="PSUM") as psf,
            tc.tile_pool(name="psg", bufs=3, space="PSUM") as psg,
        ):
            finalT = [fin.tile([P, N], F32, tag=f"fT{c}", name=f"fT{c}") for c in range(DC)]
            for dc in range(DC):
                for hf in range(2):
                    ps = psf.tile([P, D], F32, tag="psf")
                    for kc in range(DC):
                        nc.tensor.matmul(
                            ps,
                            mk(w_sb["Wo"][kc][:, dc * P:(dc + 1) * P]),
                            mk(OT[kc][:, hf * D:(hf + 1) * D]),
                            start=(kc == 0), stop=(kc == DC - 1),
                        )
                    relu = fre.tile([P, D], F32, tag="relu")
                    nc.scalar.activation(
                        out=relu, in_=ps, func=mybir.ActivationFunctionType.Relu,
                        bias=bias_t["bo"][:, dc:dc + 1], scale=1.0)
                    nc.vector.tensor_add(
                        finalT[dc][:, hf * D:(hf + 1) * D], relu,
                        OT[dc][:, hf * D:(hf + 1) * D])
            for t in range(NT):
                onat = fre.tile([P, D], F32, tag="onat")
                for dc in range(DC):
                    ps = psg.tile([P, P], F32, tag="psg")
                    nc.tensor.transpose(ps, finalT[dc][:, t * P:(t + 1) * P], ident)
                    nc.scalar.copy(onat[:, dc * P:(dc + 1) * P], ps)
                nc.sync.dma_start(out=Od.ap()[t * P:(t + 1) * P, :], in_=onat)

    nc.compile()
    return nc


def kernel(Q, K, Wq, bq, Wk, bk, Wv, bv, Wo, bo, use_f32r=True):
    key = bool(use_f32r)
    if key not in _MODULE_CACHE:
        _MODULE_CACHE[key] = build_module(use_f32r=key)
    nc = _MODULE_CACHE[key]

    Q = np.ascontiguousarray(np.asarray(Q, dtype=np.float32))
    K = np.ascontiguousarray(np.asarray(K, dtype=np.float32))
    shared = {
        name: np.ascontiguousarray(np.asarray(val, np.float32))
        for name, val in (("Wq", Wq), ("Wk", Wk), ("Wv", Wv), ("Wo", Wo),
                          ("bq", bq), ("bk", bk), ("bv", bv), ("bo", bo))
    }
    in_maps = [dict(shared, Q=Q[b], K=K[b]) for b in range(N_CORES)]
    out = run_bass_kernel_spmd(nc, in_maps, list(range(N_CORES)))
    O = np.stack([out.results[b]["O"] for b in range(N_CORES)])
    Am = np.stack([out.results[b]["Am"] for b in range(N_CORES)])
    return O, Am
